# revision 50
# baseline (speedup 1.0000x reference)
"""Multi-head attention (RoPE, causal) Trainium2 Bass kernel.

Problem: nn_MultiHeadAttention_62431644615193
  x:     [2, 2048, 1024] f32
  mask:  [1, 1, 2048, 2048] i32 (causal tril expected)
  w_qkv: [1024, 3072] f32
  w_out: [1024, 1024] f32
  out:   [2, 2048, 1024] f32

Sharding over 8 cores: data-parallel on batch (2) x tensor-parallel on
heads (16 heads -> 4 per core). Each core computes a partial output
[2048, 1024] (its heads' contribution through w_out rows); the host sums
the 4 partials per batch.

All device compute runs in bf16 (fp32 PSUM accumulation): the harness
tolerance (rel 2e-2) leaves ample room and bf16 gets full PE rate at any
moving width, half-size DMA, and fast DVE modes.

Per-core dataflow:
  1. qkT projection transposed:  qkT[c, s] = w_qk^T @ x^T   (c on partitions)
     RoPE applied via DVE stream_shuffle (adjacent-partition swap) plus a
     sign-folded sin table: qkT = raw*cos + swap(raw)*sin'.
  2. v projection natural:       v[t, e] = (x^T chunk as lhsT) @ w_v
  3. attention, scores transposed: pT[t, s] = kT_blk slice @ qT, ACT exp
     PSUM->SBUF bf16, diagonal-strip causal zeroing on DVE, PV with
     ones-augmented V giving outT[e, s] rows 0-63 plus the softmax
     denominator on row 64; normalize with reciprocal_approx_fast (DVE,
     direct from PSUM) + Pool partition_broadcast + DVE multiply.
  4. out-projection: lhsT = attn_outT chunks, rhs = w_out rows for the
     core's heads -> partial [2048, 1024]; PSUM evicted by DVE/ACT copies
     and streamed out per 128-row tile.
"""

import math

import numpy as np
import ml_dtypes

import concourse.bass as bass
import concourse.tile as tile
from concourse import bacc
import concourse.mybir as mybir
from concourse.bass_utils import run_bass_kernel_spmd

B, S, D = 2, 2048, 1024
H = 16
HD = D // H          # 64
HPC = H // 4         # 4 heads per core
ROPE_BASE = 10000.0

F32 = mybir.dt.float32
BF16 = mybir.dt.bfloat16
AF = mybir.ActivationFunctionType
NPBF = ml_dtypes.bfloat16

SWAP_MASK = [i ^ 1 for i in range(32)]


# --------------------------------------------------------------------------
# bass program (shared by all 8 cores; per-core data differs)
# --------------------------------------------------------------------------

def build_nc(causal: bool = True, reps: int = 1):
    nc = bacc.Bacc("TRN2", target_bir_lowering=False, debug=False, num_devices=8)

    xT = nc.dram_tensor("xT", [D, S], BF16, kind="ExternalInput")
    w_qk = nc.dram_tensor("w_qk", [D, 8 * HD], BF16, kind="ExternalInput")
    w_v = nc.dram_tensor("w_v", [D, 4 * HD], BF16, kind="ExternalInput")
    w_out = nc.dram_tensor("w_out", [4 * HD, D], BF16, kind="ExternalInput")
    cos2 = nc.dram_tensor("cos2", [128, S], BF16, kind="ExternalInput")
    sin2 = nc.dram_tensor("sin2", [128, S], BF16, kind="ExternalInput")
    mask01 = nc.dram_tensor("mask01", [128, 2 * 128], BF16, kind="ExternalInput")
    # bf16 partial output: halves store DMA time; host upcasts when summing
    # the per-core partials (adds ~0.2% rel err on top of ~0.5% from bf16
    # compute, well within the 2e-2 gate)
    outp = nc.dram_tensor("outp", [S, D], BF16, kind="ExternalOutput")

    NT = S // 128     # 16 t-blocks
    NI = 4            # quarter / attention s-chunks of 512

    with tile.TileContext(nc) as tc:
        with (
            tc.tile_pool(name="const", bufs=1) as cpool,
            tc.tile_pool(name="qkT", bufs=1) as qkTpool,
            tc.tile_pool(name="va", bufs=1) as vapool,
            tc.tile_pool(name="xq", bufs=2) as xqpool,
            tc.tile_pool(name="rope", bufs=1) as rppool,
            tc.tile_pool(name="phat", bufs=4) as phatpool,
            tc.tile_pool(name="norm", bufs=3) as normpool,
            tc.tile_pool(name="attn_out", bufs=8) as aopool,
            tc.tile_pool(name="outstage", bufs=3) as ostpool,
            tc.tile_pool(name="ps", bufs=1, space="PSUM") as pspool,
        ):
            # ---------------- constants ----------------
            w_qk_t = cpool.tile([128, 8, 8 * HD], BF16)
            w_v_t = cpool.tile([128, 8, 4 * HD], BF16)
            w_out_t = cpool.tile([128, 2, D], BF16)
            cos_t = cpool.tile([128, NI, 512], BF16)
            sin_t = cpool.tile([128, NI, 512], BF16)
            mask01_t = cpool.tile([128, 2, 128], BF16)

            # v_aug storage: per (t-block j, head h): [v_h(64) | 1] = 65 cols.
            # One contiguous whole-tile memset puts 1.0 in the ones columns;
            # the v parts are overwritten by the V projection before any PV
            # matmul reads them.
            va_t = vapool.tile([128, NT, 4, HD + 1], BF16)
            nc.vector.memset(va_t[:], 1.0)

            # final rotated qT/kT: [q_h0;q_h1], [k_h0;k_h1], [q_h2;q_h3], [k_h2;k_h3]
            qkT = [qkTpool.tile([128, S], BF16, name=f"qkT{i}", tag=f"qkT{i}") for i in range(4)]

            def rope_copy(src, startup=False):
                """PSUM->SBUF bf16 eviction of a raw projection tile; emitting
                it right after the matmul group frees the PSUM ring fast."""
                qk_raw = rppool.tile([128, 512], BF16, tag="qkraw", bufs=3, name="qk_raw")
                if startup:
                    nc.scalar.copy(qk_raw[:], src)
                else:
                    nc.vector.tensor_copy(qk_raw[:], src)
                return qk_raw

            def rope(mt, i, qk_raw, startup=False):
                """qkT[mt][:, chunk i] = raw*cos + swap(raw)*sin'.

                startup=True routes the trig muls to DVE (idle during the
                first projection); in steady state they go to Pool."""
                s_sl = slice(512 * i, 512 * i + 512)
                mul_eng = nc.vector if startup else nc.gpsimd
                x_swap = rppool.tile([128, 512], BF16, tag="xswap", bufs=3, name="x_swap")
                nc.vector.stream_shuffle(x_swap[:], qk_raw[:], SWAP_MASK)
                qkcos = rppool.tile([128, 512], BF16, tag="qkcos", bufs=3, name="qkcos")
                mul_eng.tensor_mul(qkcos[:], qk_raw[:], cos_t[:, i, :])
                rotsin = rppool.tile([128, 512], BF16, tag="rotsin", bufs=3, name="rotsin")
                mul_eng.tensor_mul(rotsin[:], x_swap[:], sin_t[:, i, :])
                nc.vector.tensor_add(qkT[mt][:, s_sl], qkcos[:], rotsin[:])

            def emit_loads(i, xq_box):
                s_sl = slice(512 * i, 512 * i + 512)
                xq = xqpool.tile([128, 8, 512], BF16, tag="xq", name="xq")
                xq_box[0] = xq
                nc.sync.dma_start(
                    xq[:], xT[:, s_sl].rearrange("(o p) c -> p o c", p=128)
                )
                nc.sync.dma_start(cos_t[:, i, :], cos2[:, s_sl])
                nc.sync.dma_start(sin_t[:, i, :], sin2[:, s_sl])

            def proj_chunk0():
                """Startup chunk: interleaved x/w loads, mm1 into the (still
                free) qk-tag PSUM ring, dd-outer so matmuls track DMA arrival."""
                # weights go through SWDGE (gpsimd) so their descriptor
                # generation runs in parallel with the HWDGE x loads
                xq = xqpool.tile([128, 8, 512], BF16, tag="xq", name="xq")
                for dd in range(8):
                    nc.sync.dma_start(xq[:, dd, :], xT[128 * dd : 128 * dd + 128, 0:512])
                    nc.gpsimd.dma_start(w_qk_t[:, dd, :], w_qk[128 * dd : 128 * dd + 128, :])
                nc.sync.dma_start(cos_t[:, 0, :], cos2[:, 0:512])
                nc.sync.dma_start(sin_t[:, 0, :], sin2[:, 0:512])
                nc.sync.dma_start(w_v_t[:], w_v[:].rearrange("(o p) c -> p o c", p=128))
                nc.gpsimd.dma_start(
                    mask01_t[:], mask01[:].rearrange("p (b s) -> p b s", b=2)
                )
                nc.gpsimd.dma_start(w_out_t[:], w_out[:].rearrange("(o p) c -> p o c", p=128))

                psA = pspool.tile([128, 2, 512], F32, tag="qk", bufs=2, name="psA")
                psB = pspool.tile([128, 2, 512], F32, tag="qk", bufs=2, name="psB")
                for dd in range(8):
                    for mt in range(4):
                        ps = psA if mt < 2 else psB
                        nc.tensor.matmul(
                            ps[:, mt % 2, :],
                            w_qk_t[:, dd, 128 * mt : 128 * mt + 128],
                            xq[:, dd, :],
                            start=(dd == 0),
                            stop=(dd == 7),
                        )
                for mt in range(4):
                    ps = psA if mt < 2 else psB
                    raw = rope_copy(ps[:, mt % 2, :], startup=True)
                    rope(mt, 0, raw, startup=True)
                return xq

            def proj_closures(i, xq_box):
                """Steady-state projection for chunk i >= 1: fill-work closures
                interleaved into the previous chunk's attention. Uses the
                1-bank 'proj' PSUM ring, paced by the rope PSUM->SBUF copy."""
                cls = []
                for mt in range(4):
                    box = [None, None]

                    def c_mm_a(mt=mt, box=box):
                        ps = pspool.tile(
                            [128, 512], F32, tag=("proj" if mt % 2 == 0 else "po"),
                            bufs=1, name="psp",
                        )
                        box[0] = ps
                        for dd in range(4):
                            nc.tensor.matmul(
                                ps[:],
                                w_qk_t[:, dd, 128 * mt : 128 * mt + 128],
                                xq_box[0][:, dd, :],
                                start=(dd == 0),
                                stop=False,
                            )

                    def c_mm_b(mt=mt, box=box):
                        ps = box[0]
                        for dd in range(4, 8):
                            nc.tensor.matmul(
                                ps[:],
                                w_qk_t[:, dd, 128 * mt : 128 * mt + 128],
                                xq_box[0][:, dd, :],
                                start=False,
                                stop=(dd == 7),
                            )
                        # evict immediately so the 1-bank proj ring turns over
                        box[1] = rope_copy(ps[:])

                    def c_rope(mt=mt, box=box, i=i):
                        rope(mt, i, box[1])

                    cls += [(c_mm_a, False), (c_mm_b, False), (c_rope, True)]
                return cls

            def vproj(i, xq):
                """V projection for chunk i; runs at the chunk boundary on the
                po/proj rings (both free there), so it never waits on the
                previous attention's normalize chain."""
                for st in range(4):
                    j = 4 * i + st
                    psv = pspool.tile(
                        [128, 4 * HD], F32, tag=("po" if st % 2 == 0 else "proj"),
                        bufs=1, name="psv",
                    )
                    for dd in range(8):
                        nc.tensor.matmul(
                            psv[:],
                            xq[:, dd, 128 * st : 128 * st + 128],
                            w_v_t[:, dd, :],
                            start=(dd == 0),
                            stop=(dd == 7),
                        )
                    nc.scalar.copy(
                        va_t[:, j, :, 0:HD], psv[:].rearrange("p (h c) -> p h c", h=4)
                    )

            def outproj_closures(i, ao, dve_only=False):
                """Out-projection for chunk i. Alternates the po/proj PSUM
                rings for a 2-deep pipeline (the proj ring is free whenever
                these run). dve_only keeps ACT exp-only during attention."""
                cls = []
                for st in range(4):
                    ssl = slice(512 * i + 128 * st, 512 * i + 128 * st + 128)
                    sloc = slice(128 * st, 128 * st + 128)
                    box = [None]

                    def c_half(st=st, sloc=sloc, ssl=ssl, box=box, n2=0):
                        ostage = ostpool.tile([128, D], BF16, tag="ost", name="ostage")
                        box[0] = ostage
                        ps_o = pspool.tile([128, 512], F32, tag="po", bufs=1, name="ps_o")
                        for kk in range(2):
                            nc.tensor.matmul(
                                ps_o[:],
                                ao[kk][:, sloc],
                                w_out_t[:, kk, 0:512],
                                start=(kk == 0),
                                stop=(kk == 1),
                            )
                        nc.vector.tensor_copy(ostage[:, 0:512], ps_o[:])
                        if dve_only:
                            nc.sync.dma_start(outp[ssl, 0:512], ostage[:, 0:512])

                    def c_half2(st=st, sloc=sloc, ssl=ssl, box=box, n2=1):
                        ostage = box[0]
                        ps_o = pspool.tile([128, 512], F32, tag="proj", bufs=1, name="ps_o")
                        for kk in range(2):
                            nc.tensor.matmul(
                                ps_o[:],
                                ao[kk][:, sloc],
                                w_out_t[:, kk, 512:1024],
                                start=(kk == 0),
                                stop=(kk == 1),
                            )
                        if dve_only:
                            nc.vector.tensor_copy(ostage[:, 512:1024], ps_o[:])
                            nc.sync.dma_start(outp[ssl, 512:1024], ostage[:, 512:1024])
                        else:
                            # tail: one full-row store (one HWDGE slot on the
                            # drain-critical path instead of two)
                            nc.scalar.copy(ostage[:, 512:1024], ps_o[:])
                            nc.sync.dma_start(outp[ssl, :], ostage[:])

                    cls += [(c_half, False), (c_half2, False)]
                return cls

            def attention(i, fill, pop_delay=0):
                nblk = 4 * i + 4 if causal else NT
                ao = [
                    aopool.tile([128, 512], BF16, tag="aot", name=f"ao{hp}")
                    for hp in range(2)
                ]
                # fill work (next chunk's projection, earlier chunks'
                # out-projection) is popped evenly across the j-steps so the
                # in-order PE never waits on the exp chain; pop_delay defers
                # the first pops until fresh DMA loads have landed
                steps_left = [2 * nblk]
                delay = [pop_delay]

                def pop_one(avoid_pool):
                    for idx, (fn, pool_heavy) in enumerate(fill):
                        if avoid_pool and pool_heavy:
                            continue
                        fill.pop(idx)
                        fn()
                        return True
                    return False

                def pop_fill(extra=0, avoid_pool=False):
                    if extra:
                        # off-step pops: keep PE fed across the hp transition
                        # while the normalize chain releases the pv ring
                        for _ in range(extra):
                            if not pop_one(avoid_pool=True):
                                break
                        return
                    if delay[0] > 0:
                        delay[0] -= 1
                        steps_left[0] -= 1
                        return
                    n = max(
                        0,
                        -(-len(fill) // steps_left[0]) if steps_left[0] > 0 else len(fill),
                    )
                    for _ in range(n):
                        if not pop_one(avoid_pool):
                            break
                    steps_left[0] -= 1

                for hp in range(2):
                    qt = qkT[2 * hp]
                    kt = qkT[2 * hp + 1]
                    ps_pv_e = pspool.tile([HD + 1, 512], F32, tag="pv", bufs=2, name="ps_pv_e")
                    ps_pv_o = pspool.tile([HD + 1, 512], F32, tag="pv", bufs=2, name="ps_pv_o")

                    def reg_of(j):
                        dvr = causal and 4 * i <= j
                        o = j - 4 * i if dvr else 0
                        # columns s' < 128*o of a diagonal block are fully
                        # masked -> restrict compute to [128*o : 512]
                        return dvr, o, slice(128 * o, 512)

                    def issue_qk(j):
                        dvr, o, reg = reg_of(j)
                        ps_qk = pspool.tile([128, 2, 512], F32, tag="qk", bufs=2, name="ps_qk")
                        for sl2 in range(2):
                            hb = 64 * sl2
                            nc.tensor.matmul(
                                ps_qk[:, sl2, reg],
                                kt[hb : hb + 64, 128 * j : 128 * j + 128],
                                qt[hb : hb + 64, 512 * i + reg.start : 512 * i + 512],
                                start=True,
                                stop=True,
                            )
                        phat = phatpool.tile([128, 2, 512], BF16, tag="phat", name="phat")
                        nc.scalar.activation(
                            phat[:, :, reg],
                            ps_qk[:, :, reg],
                            AF.Exp,
                            scale=1.0 / math.sqrt(HD),
                        )
                        if dvr:
                            # zero the masked triangle in the 128-wide strip
                            # where the causal boundary crosses this block
                            strip = slice(128 * o, 128 * o + 128)
                            nc.vector.tensor_mul(
                                phat[:, :, strip],
                                phat[:, :, strip],
                                mask01_t[:],
                            )
                        return phat

                    # software pipeline: issue qk/exp for j+1 before pv of j so
                    # PE (in-order) never idles waiting for exp.
                    phats = {0: issue_qk(0)}
                    for j in range(nblk):
                        if j + 1 < nblk:
                            phats[j + 1] = issue_qk(j + 1)
                        pop_fill(avoid_pool=(j >= nblk - 2 or (hp == 1 and j < 2)))
                        _, _, reg = reg_of(j)
                        phat = phats.pop(j)
                        nc.tensor.matmul(
                            ps_pv_e[:, reg],
                            va_t[:, j, 2 * hp, :],
                            phat[:, 0, reg],
                            start=(j == 0),
                            stop=(j == nblk - 1),
                        )
                        nc.tensor.matmul(
                            ps_pv_o[:, reg],
                            va_t[:, j, 2 * hp + 1, :],
                            phat[:, 1, reg],
                            start=(j == 0),
                            stop=(j == nblk - 1),
                        )
                    # Normalize via an SBUF staging copy of the whole PV
                    # accumulator (values + den row): the single DVE copy is
                    # the only PSUM reader, so the pv ring frees ~3x sooner
                    # for the next hp, and the reciprocal runs from SBUF (the
                    # custom recip uop reading PSUM misbehaves on HW).
                    # The very last normalize feeds the tail out-projection,
                    # so it runs in 128-column slices to shorten the critical
                    # path into outproj's first stripe.
                    # The den row must reach a partition-0 SBUF tile via an
                    # ACT copy before the reciprocal: the custom recip uop is
                    # only correct for partition-0 SBUF inputs on HW. The
                    # values rows are staged to SBUF by one DVE copy, whose
                    # early completion releases the pv ring for the next hp.
                    # The very last normalize feeds the tail out-projection,
                    # so it runs in 128-column slices (no staging needed:
                    # nothing reuses the pv ring after it).
                    tailcase = causal and i == NI - 1 and hp == 1
                    pvs = (ps_pv_e, ps_pv_o)
                    if not tailcase:
                        stages = []
                        for sl2 in range(2):
                            stage = normpool.tile(
                                [HD, 512], F32, tag="stage", name="stage"
                            )
                            nc.vector.tensor_copy(stage[:], pvs[sl2][0:HD, :])
                            stages.append(stage)
                    col_slices = (
                        [slice(0, 128), slice(128, 256), slice(256, 512)]
                        if tailcase
                        else [slice(0, 512)]
                    )
                    for csl in col_slices:
                        w = csl.stop - csl.start
                        recs = []
                        for sl2 in range(2):
                            den = normpool.tile([1, 512], F32, tag="den", name="den")
                            nc.scalar.copy(den[:, 0:w], pvs[sl2][HD : HD + 1, csl])
                            rec1 = normpool.tile([1, 512], F32, tag="rec1", name="rec1")
                            nc.vector.reciprocal_approx_fast(rec1[:, 0:w], den[:, 0:w])
                            recs.append(rec1)
                        brecs = []
                        for sl2 in range(2):
                            rec = normpool.tile([64, 512], F32, tag="rec", name="rec")
                            nc.gpsimd.partition_broadcast(
                                rec[:, 0:w], recs[sl2][:, 0:w]
                            )
                            brecs.append(rec)
                        for sl2 in range(2):
                            nc.vector.tensor_mul(
                                ao[hp][64 * sl2 : 64 * sl2 + 64, csl],
                                pvs[sl2][0:HD, csl] if tailcase else stages[sl2][:, csl],
                                brecs[sl2][:, 0:w],
                            )
                    if hp == 0:
                        pop_fill(extra=3)
                while fill:
                    fill.pop(0)[0]()
                return ao

            # causal: software-pipelined per-chunk schedule. attention(i) only
            # needs kT/v for t-blocks <= chunk end, so chunk i+1's projection
            # and chunk i-1's out-projection are interleaved into attention(i)
            # as PE fill work. non-causal: attention needs the full kT/v, so
            # project everything first.
            for _rep in range(reps):
                if causal:
                    xq0 = proj_chunk0()
                    vproj(0, xq0)
                    ao_all = []
                    xq_boxes = [[None] for _ in range(NI + 1)]
                    for i in range(NI):
                        fill = []
                        if i + 1 < NI:
                            emit_loads(i + 1, xq_boxes[i + 1])
                            fill += proj_closures(i + 1, xq_boxes[i + 1])
                        if i == NI - 2:
                            # chunk 0's out-projection fills attention(2),
                            # which has projection fill to spare
                            fill += outproj_closures(0, ao_all[0], dve_only=True)
                        if i == NI - 1:
                            # the rest fill the largest attention chunk,
                            # which has no projection left
                            for k in range(1, NI - 1):
                                fill += outproj_closures(k, ao_all[k], dve_only=True)
                        ao_all.append(
                            attention(i, fill, pop_delay=3 if i + 1 < NI else 0)
                        )
                        if i + 1 < NI:
                            vproj(i + 1, xq_boxes[i + 1][0])
                    for c, _ in outproj_closures(NI - 1, ao_all[-1]):
                        c()
                else:
                    xq0 = proj_chunk0()
                    vproj(0, xq0)
                    xq_boxes = [[None] for _ in range(NI)]
                    for i in range(1, NI):
                        emit_loads(i, xq_boxes[i])
                        for c, _ in proj_closures(i, xq_boxes[i]):
                            c()
                        vproj(i, xq_boxes[i][0])
                    ao_prev = None
                    for i in range(NI):
                        fill = (
                            outproj_closures(i - 1, ao_prev)
                            if ao_prev is not None
                            else []
                        )
                        ao_prev = attention(i, fill)
                    for c, _ in outproj_closures(NI - 1, ao_prev):
                        c()

    nc.compile()
    return nc


# --------------------------------------------------------------------------
# host-side: constants, sharding, assembly
# --------------------------------------------------------------------------

def _rope_tables():
    inv_freq = 1.0 / (
        ROPE_BASE ** (np.arange(0, HD, 2, dtype=np.float32) / HD)
    )
    positions = np.arange(S, dtype=np.float32)
    freqs = np.outer(positions, inv_freq).astype(np.float32)     # [S, 32]
    emb = np.concatenate((freqs, freqs), axis=-1)                # [S, 64]
    cosT = np.cos(emb).T.astype(np.float32)                      # [64, S]
    sinT = np.sin(emb).T.astype(np.float32)
    # fold the rotate_half sign into sin: rot[2i] = -x[2i+1] -> sin'[2i] < 0
    sgn = np.where(np.arange(HD) % 2 == 0, -1.0, 1.0)[:, None].astype(np.float32)
    sinT = sinT * sgn
    cos2 = np.vstack([cosT, cosT]).astype(NPBF)                  # [128, S]
    sin2 = np.vstack([sinT, sinT]).astype(NPBF)
    return cos2, sin2


_CACHE: dict = {}


def _get_nc(causal: bool):
    key = ("nc", causal)
    if key not in _CACHE:
        _CACHE[key] = build_nc(causal)
    return _CACHE[key]


def _classify_mask(mask: np.ndarray) -> str:
    m = np.asarray(mask).reshape(S, S)
    if np.array_equal(m != 0, np.tril(np.ones((S, S), bool))):
        return "causal"
    if np.all(m != 0):
        return "full"
    return "other"


def make_in_maps(x, w_qkv, w_out):
    """Build the 8 per-core input dicts (device tensors in bf16)."""
    cos2, sin2 = _rope_tables()
    # mask01[t, b, s''] = 0 where t > s'' (strict lower triangle masked), for
    # both head slots b
    m01 = (np.arange(128)[:, None] <= np.arange(128)[None, :]).astype(np.float32)
    mask01 = np.ascontiguousarray(
        np.stack([m01, m01], axis=1).reshape(128, 256)
    ).astype(NPBF)

    w3 = np.asarray(w_qkv).reshape(D, 3, H, HD)   # [D, {q,k,v}, H, hd]
    wo = np.asarray(w_out)                        # [D, D]; rows indexed [h, hd]
    xT = [np.ascontiguousarray(np.asarray(x)[b].T).astype(NPBF) for b in range(B)]

    in_maps = []
    for c in range(8):
        b, hg = divmod(c, 4)
        hs = [4 * hg + i for i in range(HPC)]
        # w_qk cols: [q_h0, q_h1, k_h0, k_h1, q_h2, q_h3, k_h2, k_h3]
        wqk_cols = []
        for pair in range(2):
            for t in range(2):  # 0 = q, 1 = k
                for hh in (hs[2 * pair], hs[2 * pair + 1]):
                    wqk_cols.append(w3[:, t, hh, :])
        w_qk_c = np.ascontiguousarray(np.concatenate(wqk_cols, axis=1)).astype(NPBF)
        w_v_c = np.ascontiguousarray(
            np.concatenate([w3[:, 2, hh, :] for hh in hs], axis=1)
        ).astype(NPBF)  # [D, 256]
        w_out_c = np.ascontiguousarray(
            np.concatenate([wo[HD * hh : HD * hh + HD, :] for hh in hs], axis=0)
        ).astype(NPBF)  # [256, D]
        in_maps.append(
            {
                "xT": xT[b],
                "w_qk": w_qk_c,
                "w_v": w_v_c,
                "w_out": w_out_c,
                "cos2": cos2,
                "sin2": sin2,
                "mask01": mask01,
            }
        )
    return in_maps


def _reference_numpy(x, mask, w_qkv, w_out):
    """Exact fallback for non-causal, non-full masks (slow, host-side)."""
    x = np.asarray(x, np.float32)
    qkv = (x @ w_qkv).reshape(B, S, 3, H, HD)
    qkv = np.transpose(qkv, (2, 0, 3, 1, 4))
    q, k, v = qkv[0], qkv[1], qkv[2]
    inv_freq = 1.0 / (ROPE_BASE ** (np.arange(0, HD, 2, dtype=np.float32) / HD))
    positions = np.arange(S, dtype=np.float32)
    freqs = np.outer(positions, inv_freq).astype(np.float32)
    emb = np.concatenate((freqs, freqs), axis=-1)
    cos = np.cos(emb)[None, None]
    sin = np.sin(emb)[None, None]

    def rot(t):
        t1 = t[..., ::2]
        t2 = t[..., 1::2]
        return np.stack((-t2, t1), axis=-1).reshape(t.shape)

    q = q * cos + rot(q) * sin
    k = k * cos + rot(k) * sin
    attn = np.einsum("bhsd,bhtd->bhst", q, k) / math.sqrt(HD)
    m = np.asarray(mask).reshape(1, 1, S, S)
    attn = np.where(m == 0, -np.inf, attn)
    attn = attn - attn.max(-1, keepdims=True)
    np.exp(attn, out=attn)
    attn /= attn.sum(-1, keepdims=True)
    out = np.einsum("bhst,bhtd->bhsd", attn, v)
    out = np.transpose(out, (0, 2, 1, 3)).reshape(B, S, D)
    return (out @ w_out).astype(np.float32)


class Runner:
    """Cached jitted SPMD runner (mirrors bass2jax.run_bass_via_pjrt)."""

    def __init__(self, nc, n_cores: int = 8):
        import jax
        import concourse.mybir as _mybir
        from concourse import bass2jax
        from jax.experimental.shard_map import shard_map
        from jax.sharding import Mesh, PartitionSpec

        bass2jax.install_neuronx_cc_hook()
        self.jax = jax
        self.n_cores = n_cores
        self._nc = nc
        in_names, out_names, out_avals, zero_outs = [], [], [], []
        for alloc in nc.m.functions[0].allocations:
            if not isinstance(alloc, _mybir.MemoryLocationSet):
                continue
            name = alloc.memorylocations[0].name
            if alloc.kind == "ExternalInput":
                in_names.append(name)
            elif alloc.kind == "ExternalOutput":
                out_names.append(name)
                shape = tuple(alloc.tensor_shape)
                dtype = _mybir.dt.np(alloc.dtype)
                out_avals.append(jax.core.ShapedArray(shape, dtype))
                zero_outs.append(np.zeros(shape, dtype))
        self.in_names = list(in_names)
        self.out_names = out_names
        self.out_avals = out_avals
        self.zero_outs = zero_outs
        all_names = in_names + out_names

        def _body(*args):
            outs = bass2jax._bass_exec_p.bind(
                *args,
                out_avals=tuple(out_avals),
                in_names=tuple(all_names),
                out_names=tuple(out_names),
                lowering_input_output_aliases=(),
                sim_require_finite=True,
                sim_require_nnan=True,
                nc=nc,
            )
            return tuple(outs)

        devices = jax.devices()[:n_cores]
        self.mesh = Mesh(np.asarray(devices), ("core",))
        n_args = len(all_names)
        self.sharded = jax.jit(
            shard_map(
                _body,
                mesh=self.mesh,
                in_specs=(PartitionSpec("core"),) * n_args,
                out_specs=(PartitionSpec("core"),) * len(out_names),
                check_rep=False,
            )
        )

    def concat_inputs(self, in_maps):
        cols = []
        for name in self.in_names:
            if name == "partition_id":
                cols.append(
                    np.arange(self.n_cores, dtype=np.uint32).reshape(
                        self.n_cores, 1
                    )
                )
            else:
                cols.append(
                    np.concatenate([np.asarray(m[name]) for m in in_maps], axis=0)
                )
        return cols

    def device_put(self, concat_in):
        """Place concatenated inputs (and zero output buffers) on the mesh."""
        from jax.sharding import NamedSharding, PartitionSpec

        sh = NamedSharding(self.mesh, PartitionSpec("core"))
        args = concat_in + [
            np.zeros((self.n_cores * z.shape[0], *z.shape[1:]), z.dtype)
            for z in self.zero_outs
        ]
        return [self.jax.device_put(a, sh) for a in args]

    def run_dev(self, dev_args):
        return self.sharded(*dev_args)

    def make_bench(self, n_reps: int):
        """Jitted fn executing the NEFF n_reps times serially on-device."""
        import jax
        from concourse import bass2jax
        from jax.experimental.shard_map import shard_map
        from jax.sharding import Mesh, PartitionSpec

        nc = self._nc
        out_avals = self.out_avals
        all_names = self.in_names + self.out_names
        out_names = self.out_names

        def _body(*args):
            outs = None
            for _ in range(n_reps):
                outs = bass2jax._bass_exec_p.bind(
                    *args,
                    out_avals=tuple(out_avals),
                    in_names=tuple(all_names),
                    out_names=tuple(out_names),
                    lowering_input_output_aliases=(),
                    sim_require_finite=True,
                    sim_require_nnan=True,
                    nc=nc,
                )
            return tuple(outs)

        n_args = len(all_names)
        return jax.jit(
            shard_map(
                _body,
                mesh=self.mesh,
                in_specs=(PartitionSpec("core"),) * n_args,
                out_specs=(PartitionSpec("core"),) * len(out_names),
                check_rep=False,
            )
        )

    def run(self, in_maps):
        dev_args = self.device_put(self.concat_inputs(in_maps))
        out_arrs = self.sharded(*dev_args)
        outs = []
        for c in range(self.n_cores):
            outs.append(
                {
                    name: np.asarray(out_arrs[i]).reshape(
                        self.n_cores, *self.out_avals[i].shape
                    )[c]
                    for i, name in enumerate(self.out_names)
                }
            )
        return outs


def _get_runner(causal: bool) -> Runner:
    key = ("runner", causal)
    if key not in _CACHE:
        _CACHE[key] = Runner(_get_nc(causal))
    return _CACHE[key]


def run_spmd(in_maps, causal: bool = True, **kw):
    nc = _get_nc(causal)
    return run_bass_kernel_spmd(nc, in_maps, core_ids=list(range(8)), **kw)


def kernel(x, mask, w_qkv, w_out):
    kind = _classify_mask(mask)
    if kind == "other":
        return _reference_numpy(x, mask, w_qkv, w_out)
    in_maps = make_in_maps(x, w_qkv, w_out)
    res = run_spmd(in_maps, causal=(kind == "causal"))
    out = np.zeros((B, S, D), np.float32)
    for c in range(8):
        out[c // 4] += res.results[c]["outp"]
    return out


if __name__ == "__main__":
    rng = np.random.default_rng(0)
    x = rng.standard_normal((B, S, D)).astype(np.float32)
    mask = np.tril(np.ones((S, S), np.int32)).reshape(1, 1, S, S)
    w_qkv = (rng.standard_normal((D, 3 * D)) * 0.02).astype(np.float32)
    w_out = (rng.standard_normal((D, D)) * 0.02).astype(np.float32)
    got = kernel(x, mask, w_qkv, w_out)
    print("kernel ran, out shape", got.shape)


# revision 51
# speedup vs baseline: 1.0053x; 1.0053x over previous
"""Multi-head attention (RoPE, causal) Trainium2 Bass kernel.

Problem: nn_MultiHeadAttention_62431644615193
  x:     [2, 2048, 1024] f32
  mask:  [1, 1, 2048, 2048] i32 (causal tril expected)
  w_qkv: [1024, 3072] f32
  w_out: [1024, 1024] f32
  out:   [2, 2048, 1024] f32

Sharding over 8 cores: data-parallel on batch (2) x tensor-parallel on
heads (16 heads -> 4 per core). Each core computes a partial output
[2048, 1024] (its heads' contribution through w_out rows); the host sums
the 4 partials per batch.

All device compute runs in bf16 (fp32 PSUM accumulation): the harness
tolerance (rel 2e-2) leaves ample room and bf16 gets full PE rate at any
moving width, half-size DMA, and fast DVE modes.

Per-core dataflow:
  1. qkT projection transposed:  qkT[c, s] = w_qk^T @ x^T   (c on partitions)
     RoPE applied via DVE stream_shuffle (adjacent-partition swap) plus a
     sign-folded sin table: qkT = raw*cos + swap(raw)*sin'.
  2. v projection natural:       v[t, e] = (x^T chunk as lhsT) @ w_v
  3. attention, scores transposed: pT[t, s] = kT_blk slice @ qT, ACT exp
     PSUM->SBUF bf16, diagonal-strip causal zeroing on DVE, PV with
     ones-augmented V giving outT[e, s] rows 0-63 plus the softmax
     denominator on row 64; normalize with reciprocal_approx_fast (DVE,
     direct from PSUM) + Pool partition_broadcast + DVE multiply.
  4. out-projection: lhsT = attn_outT chunks, rhs = w_out rows for the
     core's heads -> partial [2048, 1024]; PSUM evicted by DVE/ACT copies
     and streamed out per 128-row tile.
"""

import math

import numpy as np
import ml_dtypes

import concourse.bass as bass
import concourse.tile as tile
from concourse import bacc
import concourse.mybir as mybir
from concourse.bass_utils import run_bass_kernel_spmd

B, S, D = 2, 2048, 1024
H = 16
HD = D // H          # 64
HPC = H // 4         # 4 heads per core
ROPE_BASE = 10000.0

F32 = mybir.dt.float32
BF16 = mybir.dt.bfloat16
AF = mybir.ActivationFunctionType
NPBF = ml_dtypes.bfloat16

SWAP_MASK = [i ^ 1 for i in range(32)]


# --------------------------------------------------------------------------
# bass program (shared by all 8 cores; per-core data differs)
# --------------------------------------------------------------------------

def build_nc(causal: bool = True, reps: int = 1):
    nc = bacc.Bacc("TRN2", target_bir_lowering=False, debug=False, num_devices=8)

    xT = nc.dram_tensor("xT", [D, S], BF16, kind="ExternalInput")
    w_qk = nc.dram_tensor("w_qk", [D, 8 * HD], BF16, kind="ExternalInput")
    w_v = nc.dram_tensor("w_v", [D, 4 * HD], BF16, kind="ExternalInput")
    w_out = nc.dram_tensor("w_out", [4 * HD, D], BF16, kind="ExternalInput")
    cos2 = nc.dram_tensor("cos2", [128, S], BF16, kind="ExternalInput")
    sin2 = nc.dram_tensor("sin2", [128, S], BF16, kind="ExternalInput")
    mask01 = nc.dram_tensor("mask01", [128, 2 * 128], BF16, kind="ExternalInput")
    # bf16 partial output: halves store DMA time; host upcasts when summing
    # the per-core partials (adds ~0.2% rel err on top of ~0.5% from bf16
    # compute, well within the 2e-2 gate)
    outp = nc.dram_tensor("outp", [S, D], BF16, kind="ExternalOutput")

    NT = S // 128     # 16 t-blocks
    NI = 4            # quarter / attention s-chunks of 512

    with tile.TileContext(nc) as tc:
        with (
            tc.tile_pool(name="const", bufs=1) as cpool,
            tc.tile_pool(name="qkT", bufs=1) as qkTpool,
            tc.tile_pool(name="va", bufs=1) as vapool,
            tc.tile_pool(name="xq", bufs=2) as xqpool,
            tc.tile_pool(name="rope", bufs=1) as rppool,
            tc.tile_pool(name="phat", bufs=4) as phatpool,
            tc.tile_pool(name="norm", bufs=3) as normpool,
            tc.tile_pool(name="attn_out", bufs=8) as aopool,
            tc.tile_pool(name="outstage", bufs=3) as ostpool,
            tc.tile_pool(name="ps", bufs=1, space="PSUM") as pspool,
        ):
            # ---------------- constants ----------------
            w_qk_t = cpool.tile([128, 8, 8 * HD], BF16)
            w_v_t = cpool.tile([128, 8, 4 * HD], BF16)
            w_out_t = cpool.tile([128, 2, D], BF16)
            cos_t = cpool.tile([128, NI, 512], BF16)
            sin_t = cpool.tile([128, NI, 512], BF16)
            mask01_t = cpool.tile([128, 2, 128], BF16)

            # v_aug storage: per (t-block j, head h): [v_h(64) | 1] = 65 cols.
            # One contiguous whole-tile memset puts 1.0 in the ones columns;
            # the v parts are overwritten by the V projection before any PV
            # matmul reads them.
            va_t = vapool.tile([128, NT, 4, HD + 1], BF16)
            nc.vector.memset(va_t[:], 1.0)

            # final rotated qT/kT: [q_h0;q_h1], [k_h0;k_h1], [q_h2;q_h3], [k_h2;k_h3]
            qkT = [qkTpool.tile([128, S], BF16, name=f"qkT{i}", tag=f"qkT{i}") for i in range(4)]

            def rope_copy(src, startup=False):
                """PSUM->SBUF bf16 eviction of a raw projection tile; emitting
                it right after the matmul group frees the PSUM ring fast."""
                qk_raw = rppool.tile([128, 512], BF16, tag="qkraw", bufs=3, name="qk_raw")
                if startup:
                    nc.scalar.copy(qk_raw[:], src)
                else:
                    nc.vector.tensor_copy(qk_raw[:], src)
                return qk_raw

            def rope(mt, i, qk_raw, startup=False):
                """qkT[mt][:, chunk i] = raw*cos + swap(raw)*sin'.

                startup=True routes the trig muls to DVE (idle during the
                first projection); in steady state they go to Pool."""
                s_sl = slice(512 * i, 512 * i + 512)
                mul_eng = nc.vector if startup else nc.gpsimd
                x_swap = rppool.tile([128, 512], BF16, tag="xswap", bufs=3, name="x_swap")
                nc.vector.stream_shuffle(x_swap[:], qk_raw[:], SWAP_MASK)
                qkcos = rppool.tile([128, 512], BF16, tag="qkcos", bufs=3, name="qkcos")
                mul_eng.tensor_mul(qkcos[:], qk_raw[:], cos_t[:, i, :])
                rotsin = rppool.tile([128, 512], BF16, tag="rotsin", bufs=3, name="rotsin")
                mul_eng.tensor_mul(rotsin[:], x_swap[:], sin_t[:, i, :])
                nc.vector.tensor_add(qkT[mt][:, s_sl], qkcos[:], rotsin[:])

            def emit_loads(i, xq_box):
                s_sl = slice(512 * i, 512 * i + 512)
                xq = xqpool.tile([128, 8, 512], BF16, tag="xq", name="xq")
                xq_box[0] = xq
                nc.sync.dma_start(
                    xq[:], xT[:, s_sl].rearrange("(o p) c -> p o c", p=128)
                )
                nc.sync.dma_start(cos_t[:, i, :], cos2[:, s_sl])
                nc.sync.dma_start(sin_t[:, i, :], sin2[:, s_sl])

            def proj_chunk0():
                """Startup chunk: interleaved x/w loads, mm1 into the (still
                free) qk-tag PSUM ring, dd-outer so matmuls track DMA arrival."""
                # weights go through SWDGE (gpsimd) so their descriptor
                # generation runs in parallel with the HWDGE x loads
                xq = xqpool.tile([128, 8, 512], BF16, tag="xq", name="xq")
                for dd in range(8):
                    nc.sync.dma_start(xq[:, dd, :], xT[128 * dd : 128 * dd + 128, 0:512])
                    nc.gpsimd.dma_start(w_qk_t[:, dd, :], w_qk[128 * dd : 128 * dd + 128, :])
                nc.sync.dma_start(cos_t[:, 0, :], cos2[:, 0:512])
                nc.sync.dma_start(sin_t[:, 0, :], sin2[:, 0:512])
                nc.sync.dma_start(w_v_t[:], w_v[:].rearrange("(o p) c -> p o c", p=128))
                nc.gpsimd.dma_start(
                    mask01_t[:], mask01[:].rearrange("p (b s) -> p b s", b=2)
                )
                nc.gpsimd.dma_start(w_out_t[:], w_out[:].rearrange("(o p) c -> p o c", p=128))

                psA = pspool.tile([128, 2, 512], F32, tag="qk", bufs=2, name="psA")
                psB = pspool.tile([128, 2, 512], F32, tag="qk", bufs=2, name="psB")
                for dd in range(8):
                    for mt in range(4):
                        ps = psA if mt < 2 else psB
                        nc.tensor.matmul(
                            ps[:, mt % 2, :],
                            w_qk_t[:, dd, 128 * mt : 128 * mt + 128],
                            xq[:, dd, :],
                            start=(dd == 0),
                            stop=(dd == 7),
                        )
                for mt in range(4):
                    ps = psA if mt < 2 else psB
                    raw = rope_copy(ps[:, mt % 2, :], startup=True)
                    rope(mt, 0, raw, startup=True)
                return xq

            def proj_closures(i, xq_box):
                """Steady-state projection for chunk i >= 1: fill-work closures
                interleaved into the previous chunk's attention. Uses the
                1-bank 'proj' PSUM ring, paced by the rope PSUM->SBUF copy."""
                cls = []
                for mt in range(4):
                    box = [None, None]

                    def c_mm_a(mt=mt, box=box):
                        ps = pspool.tile(
                            [128, 512], F32, tag=("proj" if mt % 2 == 0 else "po"),
                            bufs=1, name="psp",
                        )
                        box[0] = ps
                        for dd in range(4):
                            nc.tensor.matmul(
                                ps[:],
                                w_qk_t[:, dd, 128 * mt : 128 * mt + 128],
                                xq_box[0][:, dd, :],
                                start=(dd == 0),
                                stop=False,
                            )

                    def c_mm_b(mt=mt, box=box):
                        ps = box[0]
                        for dd in range(4, 8):
                            nc.tensor.matmul(
                                ps[:],
                                w_qk_t[:, dd, 128 * mt : 128 * mt + 128],
                                xq_box[0][:, dd, :],
                                start=False,
                                stop=(dd == 7),
                            )
                        # evict immediately so the 1-bank proj ring turns over
                        box[1] = rope_copy(ps[:])

                    def c_rope(mt=mt, box=box, i=i):
                        rope(mt, i, box[1])

                    cls += [(c_mm_a, False), (c_mm_b, False), (c_rope, True)]
                return cls

            def vproj(i, xq):
                """V projection for chunk i; runs at the chunk boundary on the
                po/proj rings (both free there), so it never waits on the
                previous attention's normalize chain."""
                for st in range(4):
                    j = 4 * i + st
                    psv = pspool.tile(
                        [128, 4 * HD], F32, tag=("po" if st % 2 == 0 else "proj"),
                        bufs=1, name="psv",
                    )
                    for dd in range(8):
                        nc.tensor.matmul(
                            psv[:],
                            xq[:, dd, 128 * st : 128 * st + 128],
                            w_v_t[:, dd, :],
                            start=(dd == 0),
                            stop=(dd == 7),
                        )
                    nc.scalar.copy(
                        va_t[:, j, :, 0:HD], psv[:].rearrange("p (h c) -> p h c", h=4)
                    )

            def outproj_closures(i, ao, dve_only=False):
                """Out-projection for chunk i. Alternates the po/proj PSUM
                rings for a 2-deep pipeline (the proj ring is free whenever
                these run). dve_only keeps ACT exp-only during attention."""
                cls = []
                for st in range(4):
                    ssl = slice(512 * i + 128 * st, 512 * i + 128 * st + 128)
                    sloc = slice(128 * st, 128 * st + 128)
                    box = [None]

                    def c_half(st=st, sloc=sloc, ssl=ssl, box=box, n2=0):
                        ostage = ostpool.tile([128, D], BF16, tag="ost", name="ostage")
                        box[0] = ostage
                        ps_o = pspool.tile([128, 512], F32, tag="po", bufs=1, name="ps_o")
                        for kk in range(2):
                            nc.tensor.matmul(
                                ps_o[:],
                                ao[kk][:, sloc],
                                w_out_t[:, kk, 0:512],
                                start=(kk == 0),
                                stop=(kk == 1),
                            )
                        nc.vector.tensor_copy(ostage[:, 0:512], ps_o[:])
                        if dve_only:
                            nc.sync.dma_start(outp[ssl, 0:512], ostage[:, 0:512])

                    def c_half2(st=st, sloc=sloc, ssl=ssl, box=box, n2=1):
                        ostage = box[0]
                        ps_o = pspool.tile([128, 512], F32, tag="proj", bufs=1, name="ps_o")
                        for kk in range(2):
                            nc.tensor.matmul(
                                ps_o[:],
                                ao[kk][:, sloc],
                                w_out_t[:, kk, 512:1024],
                                start=(kk == 0),
                                stop=(kk == 1),
                            )
                        if dve_only:
                            nc.vector.tensor_copy(ostage[:, 512:1024], ps_o[:])
                            nc.sync.dma_start(outp[ssl, 512:1024], ostage[:, 512:1024])
                        else:
                            # tail: one full-row store (one HWDGE slot on the
                            # drain-critical path instead of two)
                            nc.scalar.copy(ostage[:, 512:1024], ps_o[:])
                            nc.sync.dma_start(outp[ssl, :], ostage[:])

                    cls += [(c_half, False), (c_half2, False)]
                return cls

            def attention(i, fill, pop_delay=0):
                nblk = 4 * i + 4 if causal else NT
                ao = [
                    aopool.tile([128, 512], BF16, tag="aot", name=f"ao{hp}")
                    for hp in range(2)
                ]
                # fill work (next chunk's projection, earlier chunks'
                # out-projection) is popped evenly across the j-steps so the
                # in-order PE never waits on the exp chain; pop_delay defers
                # the first pops until fresh DMA loads have landed
                steps_left = [2 * nblk]
                delay = [pop_delay]

                def pop_one(avoid_pool):
                    for idx, (fn, pool_heavy) in enumerate(fill):
                        if avoid_pool and pool_heavy:
                            continue
                        fill.pop(idx)
                        fn()
                        return True
                    return False

                def pop_fill(extra=0, avoid_pool=False):
                    if extra:
                        # off-step pops: keep PE fed across the hp transition
                        # while the normalize chain releases the pv ring
                        for _ in range(extra):
                            if not pop_one(avoid_pool=True):
                                break
                        return
                    if delay[0] > 0:
                        delay[0] -= 1
                        steps_left[0] -= 1
                        return
                    n = max(
                        0,
                        -(-len(fill) // steps_left[0]) if steps_left[0] > 0 else len(fill),
                    )
                    for _ in range(n):
                        if not pop_one(avoid_pool):
                            break
                    steps_left[0] -= 1

                for hp in range(2):
                    qt = qkT[2 * hp]
                    kt = qkT[2 * hp + 1]
                    ps_pv_e = pspool.tile([HD + 1, 512], F32, tag="pv", bufs=2, name="ps_pv_e")
                    ps_pv_o = pspool.tile([HD + 1, 512], F32, tag="pv", bufs=2, name="ps_pv_o")

                    def reg_of(j):
                        dvr = causal and 4 * i <= j
                        o = j - 4 * i if dvr else 0
                        # columns s' < 128*o of a diagonal block are fully
                        # masked -> restrict compute to [128*o : 512]
                        return dvr, o, slice(128 * o, 512)

                    def issue_qk(j):
                        dvr, o, reg = reg_of(j)
                        ps_qk = pspool.tile([128, 2, 512], F32, tag="qk", bufs=2, name="ps_qk")
                        for sl2 in range(2):
                            hb = 64 * sl2
                            nc.tensor.matmul(
                                ps_qk[:, sl2, reg],
                                kt[hb : hb + 64, 128 * j : 128 * j + 128],
                                qt[hb : hb + 64, 512 * i + reg.start : 512 * i + 512],
                                start=True,
                                stop=True,
                            )
                        phat = phatpool.tile([128, 2, 512], BF16, tag="phat", name="phat")
                        nc.scalar.activation(
                            phat[:, :, reg],
                            ps_qk[:, :, reg],
                            AF.Exp,
                            scale=1.0 / math.sqrt(HD),
                        )
                        if dvr:
                            # zero the masked triangle in the 128-wide strip
                            # where the causal boundary crosses this block
                            strip = slice(128 * o, 128 * o + 128)
                            nc.vector.tensor_mul(
                                phat[:, :, strip],
                                phat[:, :, strip],
                                mask01_t[:],
                            )
                        return phat

                    # software pipeline: issue qk/exp for j+1 before pv of j so
                    # PE (in-order) never idles waiting for exp.
                    phats = {0: issue_qk(0)}
                    for j in range(nblk):
                        if j + 1 < nblk:
                            phats[j + 1] = issue_qk(j + 1)
                        pop_fill(avoid_pool=(j >= nblk - 2 or (hp == 1 and j < 2)))
                        _, _, reg = reg_of(j)
                        phat = phats.pop(j)
                        nc.tensor.matmul(
                            ps_pv_e[:, reg],
                            va_t[:, j, 2 * hp, :],
                            phat[:, 0, reg],
                            start=(j == 0),
                            stop=(j == nblk - 1),
                        )
                        nc.tensor.matmul(
                            ps_pv_o[:, reg],
                            va_t[:, j, 2 * hp + 1, :],
                            phat[:, 1, reg],
                            start=(j == 0),
                            stop=(j == nblk - 1),
                        )
                    # Normalize via an SBUF staging copy of the whole PV
                    # accumulator (values + den row): the single DVE copy is
                    # the only PSUM reader, so the pv ring frees ~3x sooner
                    # for the next hp, and the reciprocal runs from SBUF (the
                    # custom recip uop reading PSUM misbehaves on HW).
                    # The very last normalize feeds the tail out-projection,
                    # so it runs in 128-column slices to shorten the critical
                    # path into outproj's first stripe.
                    # The den row must reach a partition-0 SBUF tile via an
                    # ACT copy before the reciprocal: the custom recip uop is
                    # only correct for partition-0 SBUF inputs on HW. The
                    # values rows are staged to SBUF by one DVE copy, whose
                    # early completion releases the pv ring for the next hp.
                    # The very last normalize feeds the tail out-projection,
                    # so it runs in 128-column slices (no staging needed:
                    # nothing reuses the pv ring after it).
                    tailcase = causal and i == NI - 1 and hp == 1
                    pvs = (ps_pv_e, ps_pv_o)
                    if not tailcase:
                        stages = []
                        for sl2 in range(2):
                            stage = normpool.tile(
                                [HD, 512], F32, tag="stage", name="stage"
                            )
                            nc.vector.tensor_copy(stage[:], pvs[sl2][0:HD, :])
                            stages.append(stage)
                    col_slices = (
                        [slice(0, 256), slice(256, 512)]
                        if tailcase
                        else [slice(0, 512)]
                    )
                    for csl in col_slices:
                        w = csl.stop - csl.start
                        recs = []
                        for sl2 in range(2):
                            den = normpool.tile([1, 512], F32, tag="den", name="den")
                            nc.scalar.copy(den[:, 0:w], pvs[sl2][HD : HD + 1, csl])
                            rec1 = normpool.tile([1, 512], F32, tag="rec1", name="rec1")
                            nc.vector.reciprocal_approx_fast(rec1[:, 0:w], den[:, 0:w])
                            recs.append(rec1)
                        brecs = []
                        for sl2 in range(2):
                            rec = normpool.tile([64, 512], F32, tag="rec", name="rec")
                            nc.gpsimd.partition_broadcast(
                                rec[:, 0:w], recs[sl2][:, 0:w]
                            )
                            brecs.append(rec)
                        for sl2 in range(2):
                            nc.vector.tensor_mul(
                                ao[hp][64 * sl2 : 64 * sl2 + 64, csl],
                                pvs[sl2][0:HD, csl] if tailcase else stages[sl2][:, csl],
                                brecs[sl2][:, 0:w],
                            )
                    if hp == 0:
                        pop_fill(extra=2)
                while fill:
                    fill.pop(0)[0]()
                return ao

            # causal: software-pipelined per-chunk schedule. attention(i) only
            # needs kT/v for t-blocks <= chunk end, so chunk i+1's projection
            # and chunk i-1's out-projection are interleaved into attention(i)
            # as PE fill work. non-causal: attention needs the full kT/v, so
            # project everything first.
            for _rep in range(reps):
                if causal:
                    xq0 = proj_chunk0()
                    vproj(0, xq0)
                    ao_all = []
                    xq_boxes = [[None] for _ in range(NI + 1)]
                    for i in range(NI):
                        fill = []
                        if i + 1 < NI:
                            emit_loads(i + 1, xq_boxes[i + 1])
                            fill += proj_closures(i + 1, xq_boxes[i + 1])
                        if i == NI - 2:
                            # chunk 0's out-projection fills attention(2),
                            # which has projection fill to spare
                            fill += outproj_closures(0, ao_all[0], dve_only=True)
                        if i == NI - 1:
                            # the rest fill the largest attention chunk,
                            # which has no projection left
                            for k in range(1, NI - 1):
                                fill += outproj_closures(k, ao_all[k], dve_only=True)
                        ao_all.append(
                            attention(i, fill, pop_delay=3 if i + 1 < NI else 0)
                        )
                        if i + 1 < NI:
                            vproj(i + 1, xq_boxes[i + 1][0])
                    for c, _ in outproj_closures(NI - 1, ao_all[-1]):
                        c()
                else:
                    xq0 = proj_chunk0()
                    vproj(0, xq0)
                    xq_boxes = [[None] for _ in range(NI)]
                    for i in range(1, NI):
                        emit_loads(i, xq_boxes[i])
                        for c, _ in proj_closures(i, xq_boxes[i]):
                            c()
                        vproj(i, xq_boxes[i][0])
                    ao_prev = None
                    for i in range(NI):
                        fill = (
                            outproj_closures(i - 1, ao_prev)
                            if ao_prev is not None
                            else []
                        )
                        ao_prev = attention(i, fill)
                    for c, _ in outproj_closures(NI - 1, ao_prev):
                        c()

    nc.compile()
    return nc


# --------------------------------------------------------------------------
# host-side: constants, sharding, assembly
# --------------------------------------------------------------------------

def _rope_tables():
    inv_freq = 1.0 / (
        ROPE_BASE ** (np.arange(0, HD, 2, dtype=np.float32) / HD)
    )
    positions = np.arange(S, dtype=np.float32)
    freqs = np.outer(positions, inv_freq).astype(np.float32)     # [S, 32]
    emb = np.concatenate((freqs, freqs), axis=-1)                # [S, 64]
    cosT = np.cos(emb).T.astype(np.float32)                      # [64, S]
    sinT = np.sin(emb).T.astype(np.float32)
    # fold the rotate_half sign into sin: rot[2i] = -x[2i+1] -> sin'[2i] < 0
    sgn = np.where(np.arange(HD) % 2 == 0, -1.0, 1.0)[:, None].astype(np.float32)
    sinT = sinT * sgn
    cos2 = np.vstack([cosT, cosT]).astype(NPBF)                  # [128, S]
    sin2 = np.vstack([sinT, sinT]).astype(NPBF)
    return cos2, sin2


_CACHE: dict = {}


def _get_nc(causal: bool):
    key = ("nc", causal)
    if key not in _CACHE:
        _CACHE[key] = build_nc(causal)
    return _CACHE[key]


def _classify_mask(mask: np.ndarray) -> str:
    m = np.asarray(mask).reshape(S, S)
    if np.array_equal(m != 0, np.tril(np.ones((S, S), bool))):
        return "causal"
    if np.all(m != 0):
        return "full"
    return "other"


def make_in_maps(x, w_qkv, w_out):
    """Build the 8 per-core input dicts (device tensors in bf16)."""
    cos2, sin2 = _rope_tables()
    # mask01[t, b, s''] = 0 where t > s'' (strict lower triangle masked), for
    # both head slots b
    m01 = (np.arange(128)[:, None] <= np.arange(128)[None, :]).astype(np.float32)
    mask01 = np.ascontiguousarray(
        np.stack([m01, m01], axis=1).reshape(128, 256)
    ).astype(NPBF)

    w3 = np.asarray(w_qkv).reshape(D, 3, H, HD)   # [D, {q,k,v}, H, hd]
    wo = np.asarray(w_out)                        # [D, D]; rows indexed [h, hd]
    xT = [np.ascontiguousarray(np.asarray(x)[b].T).astype(NPBF) for b in range(B)]

    in_maps = []
    for c in range(8):
        b, hg = divmod(c, 4)
        hs = [4 * hg + i for i in range(HPC)]
        # w_qk cols: [q_h0, q_h1, k_h0, k_h1, q_h2, q_h3, k_h2, k_h3]
        wqk_cols = []
        for pair in range(2):
            for t in range(2):  # 0 = q, 1 = k
                for hh in (hs[2 * pair], hs[2 * pair + 1]):
                    wqk_cols.append(w3[:, t, hh, :])
        w_qk_c = np.ascontiguousarray(np.concatenate(wqk_cols, axis=1)).astype(NPBF)
        w_v_c = np.ascontiguousarray(
            np.concatenate([w3[:, 2, hh, :] for hh in hs], axis=1)
        ).astype(NPBF)  # [D, 256]
        w_out_c = np.ascontiguousarray(
            np.concatenate([wo[HD * hh : HD * hh + HD, :] for hh in hs], axis=0)
        ).astype(NPBF)  # [256, D]
        in_maps.append(
            {
                "xT": xT[b],
                "w_qk": w_qk_c,
                "w_v": w_v_c,
                "w_out": w_out_c,
                "cos2": cos2,
                "sin2": sin2,
                "mask01": mask01,
            }
        )
    return in_maps


def _reference_numpy(x, mask, w_qkv, w_out):
    """Exact fallback for non-causal, non-full masks (slow, host-side)."""
    x = np.asarray(x, np.float32)
    qkv = (x @ w_qkv).reshape(B, S, 3, H, HD)
    qkv = np.transpose(qkv, (2, 0, 3, 1, 4))
    q, k, v = qkv[0], qkv[1], qkv[2]
    inv_freq = 1.0 / (ROPE_BASE ** (np.arange(0, HD, 2, dtype=np.float32) / HD))
    positions = np.arange(S, dtype=np.float32)
    freqs = np.outer(positions, inv_freq).astype(np.float32)
    emb = np.concatenate((freqs, freqs), axis=-1)
    cos = np.cos(emb)[None, None]
    sin = np.sin(emb)[None, None]

    def rot(t):
        t1 = t[..., ::2]
        t2 = t[..., 1::2]
        return np.stack((-t2, t1), axis=-1).reshape(t.shape)

    q = q * cos + rot(q) * sin
    k = k * cos + rot(k) * sin
    attn = np.einsum("bhsd,bhtd->bhst", q, k) / math.sqrt(HD)
    m = np.asarray(mask).reshape(1, 1, S, S)
    attn = np.where(m == 0, -np.inf, attn)
    attn = attn - attn.max(-1, keepdims=True)
    np.exp(attn, out=attn)
    attn /= attn.sum(-1, keepdims=True)
    out = np.einsum("bhst,bhtd->bhsd", attn, v)
    out = np.transpose(out, (0, 2, 1, 3)).reshape(B, S, D)
    return (out @ w_out).astype(np.float32)


class Runner:
    """Cached jitted SPMD runner (mirrors bass2jax.run_bass_via_pjrt)."""

    def __init__(self, nc, n_cores: int = 8):
        import jax
        import concourse.mybir as _mybir
        from concourse import bass2jax
        from jax.experimental.shard_map import shard_map
        from jax.sharding import Mesh, PartitionSpec

        bass2jax.install_neuronx_cc_hook()
        self.jax = jax
        self.n_cores = n_cores
        self._nc = nc
        in_names, out_names, out_avals, zero_outs = [], [], [], []
        for alloc in nc.m.functions[0].allocations:
            if not isinstance(alloc, _mybir.MemoryLocationSet):
                continue
            name = alloc.memorylocations[0].name
            if alloc.kind == "ExternalInput":
                in_names.append(name)
            elif alloc.kind == "ExternalOutput":
                out_names.append(name)
                shape = tuple(alloc.tensor_shape)
                dtype = _mybir.dt.np(alloc.dtype)
                out_avals.append(jax.core.ShapedArray(shape, dtype))
                zero_outs.append(np.zeros(shape, dtype))
        self.in_names = list(in_names)
        self.out_names = out_names
        self.out_avals = out_avals
        self.zero_outs = zero_outs
        all_names = in_names + out_names

        def _body(*args):
            outs = bass2jax._bass_exec_p.bind(
                *args,
                out_avals=tuple(out_avals),
                in_names=tuple(all_names),
                out_names=tuple(out_names),
                lowering_input_output_aliases=(),
                sim_require_finite=True,
                sim_require_nnan=True,
                nc=nc,
            )
            return tuple(outs)

        devices = jax.devices()[:n_cores]
        self.mesh = Mesh(np.asarray(devices), ("core",))
        n_args = len(all_names)
        self.sharded = jax.jit(
            shard_map(
                _body,
                mesh=self.mesh,
                in_specs=(PartitionSpec("core"),) * n_args,
                out_specs=(PartitionSpec("core"),) * len(out_names),
                check_rep=False,
            )
        )

    def concat_inputs(self, in_maps):
        cols = []
        for name in self.in_names:
            if name == "partition_id":
                cols.append(
                    np.arange(self.n_cores, dtype=np.uint32).reshape(
                        self.n_cores, 1
                    )
                )
            else:
                cols.append(
                    np.concatenate([np.asarray(m[name]) for m in in_maps], axis=0)
                )
        return cols

    def device_put(self, concat_in):
        """Place concatenated inputs (and zero output buffers) on the mesh."""
        from jax.sharding import NamedSharding, PartitionSpec

        sh = NamedSharding(self.mesh, PartitionSpec("core"))
        args = concat_in + [
            np.zeros((self.n_cores * z.shape[0], *z.shape[1:]), z.dtype)
            for z in self.zero_outs
        ]
        return [self.jax.device_put(a, sh) for a in args]

    def run_dev(self, dev_args):
        return self.sharded(*dev_args)

    def make_bench(self, n_reps: int):
        """Jitted fn executing the NEFF n_reps times serially on-device."""
        import jax
        from concourse import bass2jax
        from jax.experimental.shard_map import shard_map
        from jax.sharding import Mesh, PartitionSpec

        nc = self._nc
        out_avals = self.out_avals
        all_names = self.in_names + self.out_names
        out_names = self.out_names

        def _body(*args):
            outs = None
            for _ in range(n_reps):
                outs = bass2jax._bass_exec_p.bind(
                    *args,
                    out_avals=tuple(out_avals),
                    in_names=tuple(all_names),
                    out_names=tuple(out_names),
                    lowering_input_output_aliases=(),
                    sim_require_finite=True,
                    sim_require_nnan=True,
                    nc=nc,
                )
            return tuple(outs)

        n_args = len(all_names)
        return jax.jit(
            shard_map(
                _body,
                mesh=self.mesh,
                in_specs=(PartitionSpec("core"),) * n_args,
                out_specs=(PartitionSpec("core"),) * len(out_names),
                check_rep=False,
            )
        )

    def run(self, in_maps):
        dev_args = self.device_put(self.concat_inputs(in_maps))
        out_arrs = self.sharded(*dev_args)
        outs = []
        for c in range(self.n_cores):
            outs.append(
                {
                    name: np.asarray(out_arrs[i]).reshape(
                        self.n_cores, *self.out_avals[i].shape
                    )[c]
                    for i, name in enumerate(self.out_names)
                }
            )
        return outs


def _get_runner(causal: bool) -> Runner:
    key = ("runner", causal)
    if key not in _CACHE:
        _CACHE[key] = Runner(_get_nc(causal))
    return _CACHE[key]


def run_spmd(in_maps, causal: bool = True, **kw):
    nc = _get_nc(causal)
    return run_bass_kernel_spmd(nc, in_maps, core_ids=list(range(8)), **kw)


def kernel(x, mask, w_qkv, w_out):
    kind = _classify_mask(mask)
    if kind == "other":
        return _reference_numpy(x, mask, w_qkv, w_out)
    in_maps = make_in_maps(x, w_qkv, w_out)
    res = run_spmd(in_maps, causal=(kind == "causal"))
    out = np.zeros((B, S, D), np.float32)
    for c in range(8):
        out[c // 4] += res.results[c]["outp"]
    return out


if __name__ == "__main__":
    rng = np.random.default_rng(0)
    x = rng.standard_normal((B, S, D)).astype(np.float32)
    mask = np.tril(np.ones((S, S), np.int32)).reshape(1, 1, S, S)
    w_qkv = (rng.standard_normal((D, 3 * D)) * 0.02).astype(np.float32)
    w_out = (rng.standard_normal((D, D)) * 0.02).astype(np.float32)
    got = kernel(x, mask, w_qkv, w_out)
    print("kernel ran, out shape", got.shape)


# revision 52
# speedup vs baseline: 1.0057x; 1.0004x over previous
"""Multi-head attention (RoPE, causal) Trainium2 Bass kernel.

Problem: nn_MultiHeadAttention_62431644615193
  x:     [2, 2048, 1024] f32
  mask:  [1, 1, 2048, 2048] i32 (causal tril expected)
  w_qkv: [1024, 3072] f32
  w_out: [1024, 1024] f32
  out:   [2, 2048, 1024] f32

Sharding over 8 cores: data-parallel on batch (2) x tensor-parallel on
heads (16 heads -> 4 per core). Each core computes a partial output
[2048, 1024] (its heads' contribution through w_out rows); the host sums
the 4 partials per batch.

All device compute runs in bf16 (fp32 PSUM accumulation): the harness
tolerance (rel 2e-2) leaves ample room and bf16 gets full PE rate at any
moving width, half-size DMA, and fast DVE modes.

Per-core dataflow:
  1. qkT projection transposed:  qkT[c, s] = w_qk^T @ x^T   (c on partitions)
     RoPE applied via DVE stream_shuffle (adjacent-partition swap) plus a
     sign-folded sin table: qkT = raw*cos + swap(raw)*sin'.
  2. v projection natural:       v[t, e] = (x^T chunk as lhsT) @ w_v
  3. attention, scores transposed: pT[t, s] = kT_blk slice @ qT, ACT exp
     PSUM->SBUF bf16, diagonal-strip causal zeroing on DVE, PV with
     ones-augmented V giving outT[e, s] rows 0-63 plus the softmax
     denominator on row 64; normalize with reciprocal_approx_fast (DVE,
     direct from PSUM) + Pool partition_broadcast + DVE multiply.
  4. out-projection: lhsT = attn_outT chunks, rhs = w_out rows for the
     core's heads -> partial [2048, 1024]; PSUM evicted by DVE/ACT copies
     and streamed out per 128-row tile.
"""

import math

import numpy as np
import ml_dtypes

import concourse.bass as bass
import concourse.tile as tile
from concourse import bacc
import concourse.mybir as mybir
from concourse.bass_utils import run_bass_kernel_spmd

B, S, D = 2, 2048, 1024
H = 16
HD = D // H          # 64
HPC = H // 4         # 4 heads per core
ROPE_BASE = 10000.0

F32 = mybir.dt.float32
BF16 = mybir.dt.bfloat16
AF = mybir.ActivationFunctionType
NPBF = ml_dtypes.bfloat16

SWAP_MASK = [i ^ 1 for i in range(32)]


# --------------------------------------------------------------------------
# bass program (shared by all 8 cores; per-core data differs)
# --------------------------------------------------------------------------

def build_nc(causal: bool = True, reps: int = 1):
    nc = bacc.Bacc("TRN2", target_bir_lowering=False, debug=False, num_devices=8)

    xT = nc.dram_tensor("xT", [D, S], BF16, kind="ExternalInput")
    w_qk = nc.dram_tensor("w_qk", [D, 8 * HD], BF16, kind="ExternalInput")
    w_v = nc.dram_tensor("w_v", [D, 4 * HD], BF16, kind="ExternalInput")
    w_out = nc.dram_tensor("w_out", [4 * HD, D], BF16, kind="ExternalInput")
    cos2 = nc.dram_tensor("cos2", [128, S], BF16, kind="ExternalInput")
    sin2 = nc.dram_tensor("sin2", [128, S], BF16, kind="ExternalInput")
    mask01 = nc.dram_tensor("mask01", [128, 2 * 128], BF16, kind="ExternalInput")
    # bf16 partial output: halves store DMA time; host upcasts when summing
    # the per-core partials (adds ~0.2% rel err on top of ~0.5% from bf16
    # compute, well within the 2e-2 gate)
    outp = nc.dram_tensor("outp", [S, D], BF16, kind="ExternalOutput")

    NT = S // 128     # 16 t-blocks
    NI = 4            # quarter / attention s-chunks of 512

    with tile.TileContext(nc) as tc:
        with (
            tc.tile_pool(name="const", bufs=1) as cpool,
            tc.tile_pool(name="qkT", bufs=1) as qkTpool,
            tc.tile_pool(name="va", bufs=1) as vapool,
            tc.tile_pool(name="xq", bufs=2) as xqpool,
            tc.tile_pool(name="rope", bufs=1) as rppool,
            tc.tile_pool(name="phat", bufs=4) as phatpool,
            tc.tile_pool(name="norm", bufs=3) as normpool,
            tc.tile_pool(name="attn_out", bufs=8) as aopool,
            tc.tile_pool(name="outstage", bufs=3) as ostpool,
            tc.tile_pool(name="ps", bufs=1, space="PSUM") as pspool,
        ):
            # ---------------- constants ----------------
            w_qk_t = cpool.tile([128, 8, 8 * HD], BF16)
            w_v_t = cpool.tile([128, 8, 4 * HD], BF16)
            w_out_t = cpool.tile([128, 2, D], BF16)
            cos_t = cpool.tile([128, NI, 512], BF16)
            sin_t = cpool.tile([128, NI, 512], BF16)
            mask01_t = cpool.tile([128, 2, 128], BF16)

            # v_aug storage: per (t-block j, head h): [v_h(64) | 1] = 65 cols.
            # One contiguous whole-tile memset puts 1.0 in the ones columns;
            # the v parts are overwritten by the V projection before any PV
            # matmul reads them.
            va_t = vapool.tile([128, NT, 4, HD + 1], BF16)
            nc.vector.memset(va_t[:], 1.0)

            # final rotated qT/kT: [q_h0;q_h1], [k_h0;k_h1], [q_h2;q_h3], [k_h2;k_h3]
            qkT = [qkTpool.tile([128, S], BF16, name=f"qkT{i}", tag=f"qkT{i}") for i in range(4)]

            def rope_copy(src, startup=False):
                """PSUM->SBUF bf16 eviction of a raw projection tile; emitting
                it right after the matmul group frees the PSUM ring fast."""
                qk_raw = rppool.tile([128, 512], BF16, tag="qkraw", bufs=3, name="qk_raw")
                if startup:
                    nc.scalar.copy(qk_raw[:], src)
                else:
                    nc.vector.tensor_copy(qk_raw[:], src)
                return qk_raw

            def rope(mt, i, qk_raw, startup=False):
                """qkT[mt][:, chunk i] = raw*cos + swap(raw)*sin'.

                startup=True routes the trig muls to DVE (idle during the
                first projection); in steady state they go to Pool."""
                s_sl = slice(512 * i, 512 * i + 512)
                mul_eng = nc.vector if startup else nc.gpsimd
                x_swap = rppool.tile([128, 512], BF16, tag="xswap", bufs=3, name="x_swap")
                nc.vector.stream_shuffle(x_swap[:], qk_raw[:], SWAP_MASK)
                qkcos = rppool.tile([128, 512], BF16, tag="qkcos", bufs=3, name="qkcos")
                mul_eng.tensor_mul(qkcos[:], qk_raw[:], cos_t[:, i, :])
                rotsin = rppool.tile([128, 512], BF16, tag="rotsin", bufs=3, name="rotsin")
                mul_eng.tensor_mul(rotsin[:], x_swap[:], sin_t[:, i, :])
                nc.vector.tensor_add(qkT[mt][:, s_sl], qkcos[:], rotsin[:])

            def emit_loads(i, xq_box):
                s_sl = slice(512 * i, 512 * i + 512)
                xq = xqpool.tile([128, 8, 512], BF16, tag="xq", name="xq")
                xq_box[0] = xq
                nc.sync.dma_start(
                    xq[:], xT[:, s_sl].rearrange("(o p) c -> p o c", p=128)
                )
                nc.sync.dma_start(cos_t[:, i, :], cos2[:, s_sl])
                nc.sync.dma_start(sin_t[:, i, :], sin2[:, s_sl])

            def proj_chunk0():
                """Startup chunk: interleaved x/w loads, mm1 into the (still
                free) qk-tag PSUM ring, dd-outer so matmuls track DMA arrival."""
                # weights go through SWDGE (gpsimd) so their descriptor
                # generation runs in parallel with the HWDGE x loads
                xq = xqpool.tile([128, 8, 512], BF16, tag="xq", name="xq")
                for dd in range(8):
                    nc.sync.dma_start(xq[:, dd, :], xT[128 * dd : 128 * dd + 128, 0:512])
                    nc.gpsimd.dma_start(w_qk_t[:, dd, :], w_qk[128 * dd : 128 * dd + 128, :])
                nc.sync.dma_start(cos_t[:, 0, :], cos2[:, 0:512])
                nc.sync.dma_start(sin_t[:, 0, :], sin2[:, 0:512])
                nc.sync.dma_start(w_v_t[:], w_v[:].rearrange("(o p) c -> p o c", p=128))
                nc.gpsimd.dma_start(
                    mask01_t[:], mask01[:].rearrange("p (b s) -> p b s", b=2)
                )
                nc.gpsimd.dma_start(w_out_t[:], w_out[:].rearrange("(o p) c -> p o c", p=128))

                psA = pspool.tile([128, 2, 512], F32, tag="qk", bufs=2, name="psA")
                psB = pspool.tile([128, 2, 512], F32, tag="qk", bufs=2, name="psB")
                for dd in range(8):
                    for mt in range(4):
                        ps = psA if mt < 2 else psB
                        nc.tensor.matmul(
                            ps[:, mt % 2, :],
                            w_qk_t[:, dd, 128 * mt : 128 * mt + 128],
                            xq[:, dd, :],
                            start=(dd == 0),
                            stop=(dd == 7),
                        )
                for mt in range(4):
                    ps = psA if mt < 2 else psB
                    raw = rope_copy(ps[:, mt % 2, :], startup=True)
                    rope(mt, 0, raw, startup=True)
                return xq

            def proj_closures(i, xq_box):
                """Steady-state projection for chunk i >= 1: fill-work closures
                interleaved into the previous chunk's attention. Uses the
                1-bank 'proj' PSUM ring, paced by the rope PSUM->SBUF copy."""
                cls = []
                for mt in range(4):
                    box = [None, None]

                    def c_mm_a(mt=mt, box=box):
                        ps = pspool.tile(
                            [128, 512], F32, tag=("proj" if mt % 2 == 0 else "po"),
                            bufs=1, name="psp",
                        )
                        box[0] = ps
                        for dd in range(4):
                            nc.tensor.matmul(
                                ps[:],
                                w_qk_t[:, dd, 128 * mt : 128 * mt + 128],
                                xq_box[0][:, dd, :],
                                start=(dd == 0),
                                stop=False,
                            )

                    def c_mm_b(mt=mt, box=box):
                        ps = box[0]
                        for dd in range(4, 8):
                            nc.tensor.matmul(
                                ps[:],
                                w_qk_t[:, dd, 128 * mt : 128 * mt + 128],
                                xq_box[0][:, dd, :],
                                start=False,
                                stop=(dd == 7),
                            )
                        # evict immediately so the 1-bank proj ring turns over
                        box[1] = rope_copy(ps[:])

                    def c_rope(mt=mt, box=box, i=i):
                        rope(mt, i, box[1])

                    cls += [(c_mm_a, False), (c_mm_b, False), (c_rope, True)]
                return cls

            def vproj(i, xq):
                """V projection for chunk i; runs at the chunk boundary on the
                po/proj rings (both free there), so it never waits on the
                previous attention's normalize chain."""
                for st in range(4):
                    j = 4 * i + st
                    psv = pspool.tile(
                        [128, 4 * HD], F32, tag=("proj" if st % 2 == 0 else "po"),
                        bufs=1, name="psv",
                    )
                    for dd in range(8):
                        nc.tensor.matmul(
                            psv[:],
                            xq[:, dd, 128 * st : 128 * st + 128],
                            w_v_t[:, dd, :],
                            start=(dd == 0),
                            stop=(dd == 7),
                        )
                    nc.scalar.copy(
                        va_t[:, j, :, 0:HD], psv[:].rearrange("p (h c) -> p h c", h=4)
                    )

            def outproj_closures(i, ao, dve_only=False):
                """Out-projection for chunk i. Alternates the po/proj PSUM
                rings for a 2-deep pipeline (the proj ring is free whenever
                these run). dve_only keeps ACT exp-only during attention."""
                cls = []
                for st in range(4):
                    ssl = slice(512 * i + 128 * st, 512 * i + 128 * st + 128)
                    sloc = slice(128 * st, 128 * st + 128)
                    box = [None]

                    def c_half(st=st, sloc=sloc, ssl=ssl, box=box, n2=0):
                        ostage = ostpool.tile([128, D], BF16, tag="ost", name="ostage")
                        box[0] = ostage
                        ps_o = pspool.tile([128, 512], F32, tag="po", bufs=1, name="ps_o")
                        for kk in range(2):
                            nc.tensor.matmul(
                                ps_o[:],
                                ao[kk][:, sloc],
                                w_out_t[:, kk, 0:512],
                                start=(kk == 0),
                                stop=(kk == 1),
                            )
                        nc.vector.tensor_copy(ostage[:, 0:512], ps_o[:])
                        nc.sync.dma_start(outp[ssl, 0:512], ostage[:, 0:512])

                    def c_half2(st=st, sloc=sloc, ssl=ssl, box=box, n2=1):
                        ostage = box[0]
                        ps_o = pspool.tile([128, 512], F32, tag="proj", bufs=1, name="ps_o")
                        for kk in range(2):
                            nc.tensor.matmul(
                                ps_o[:],
                                ao[kk][:, sloc],
                                w_out_t[:, kk, 512:1024],
                                start=(kk == 0),
                                stop=(kk == 1),
                            )
                        if dve_only:
                            nc.vector.tensor_copy(ostage[:, 512:1024], ps_o[:])
                        else:
                            nc.scalar.copy(ostage[:, 512:1024], ps_o[:])
                        nc.sync.dma_start(outp[ssl, 512:1024], ostage[:, 512:1024])

                    cls += [(c_half, False), (c_half2, False)]
                return cls

            def attention(i, fill, pop_delay=0):
                nblk = 4 * i + 4 if causal else NT
                ao = [
                    aopool.tile([128, 512], BF16, tag="aot", name=f"ao{hp}")
                    for hp in range(2)
                ]
                # fill work (next chunk's projection, earlier chunks'
                # out-projection) is popped evenly across the j-steps so the
                # in-order PE never waits on the exp chain; pop_delay defers
                # the first pops until fresh DMA loads have landed
                steps_left = [2 * nblk]
                delay = [pop_delay]

                def pop_one(avoid_pool):
                    for idx, (fn, pool_heavy) in enumerate(fill):
                        if avoid_pool and pool_heavy:
                            continue
                        fill.pop(idx)
                        fn()
                        return True
                    return False

                def pop_fill(extra=0, avoid_pool=False):
                    if extra:
                        # off-step pops: keep PE fed across the hp transition
                        # while the normalize chain releases the pv ring
                        for _ in range(extra):
                            if not pop_one(avoid_pool=True):
                                break
                        return
                    if delay[0] > 0:
                        delay[0] -= 1
                        steps_left[0] -= 1
                        return
                    n = max(
                        0,
                        -(-len(fill) // steps_left[0]) if steps_left[0] > 0 else len(fill),
                    )
                    for _ in range(n):
                        if not pop_one(avoid_pool):
                            break
                    steps_left[0] -= 1

                for hp in range(2):
                    qt = qkT[2 * hp]
                    kt = qkT[2 * hp + 1]
                    ps_pv_e = pspool.tile([HD + 1, 512], F32, tag="pv", bufs=2, name="ps_pv_e")
                    ps_pv_o = pspool.tile([HD + 1, 512], F32, tag="pv", bufs=2, name="ps_pv_o")

                    def reg_of(j):
                        dvr = causal and 4 * i <= j
                        o = j - 4 * i if dvr else 0
                        # columns s' < 128*o of a diagonal block are fully
                        # masked -> restrict compute to [128*o : 512]
                        return dvr, o, slice(128 * o, 512)

                    def issue_qk(j):
                        dvr, o, reg = reg_of(j)
                        ps_qk = pspool.tile([128, 2, 512], F32, tag="qk", bufs=2, name="ps_qk")
                        for sl2 in range(2):
                            hb = 64 * sl2
                            nc.tensor.matmul(
                                ps_qk[:, sl2, reg],
                                kt[hb : hb + 64, 128 * j : 128 * j + 128],
                                qt[hb : hb + 64, 512 * i + reg.start : 512 * i + 512],
                                start=True,
                                stop=True,
                            )
                        phat = phatpool.tile([128, 2, 512], BF16, tag="phat", name="phat")
                        nc.scalar.activation(
                            phat[:, :, reg],
                            ps_qk[:, :, reg],
                            AF.Exp,
                            scale=1.0 / math.sqrt(HD),
                        )
                        if dvr:
                            # zero the masked triangle in the 128-wide strip
                            # where the causal boundary crosses this block
                            strip = slice(128 * o, 128 * o + 128)
                            nc.vector.tensor_mul(
                                phat[:, :, strip],
                                phat[:, :, strip],
                                mask01_t[:],
                            )
                        return phat

                    # software pipeline: issue qk/exp for j+1 before pv of j so
                    # PE (in-order) never idles waiting for exp.
                    phats = {0: issue_qk(0)}
                    for j in range(nblk):
                        if j + 1 < nblk:
                            phats[j + 1] = issue_qk(j + 1)
                        pop_fill(avoid_pool=(j >= nblk - 2 or (hp == 1 and j < 2)))
                        _, _, reg = reg_of(j)
                        phat = phats.pop(j)
                        nc.tensor.matmul(
                            ps_pv_e[:, reg],
                            va_t[:, j, 2 * hp, :],
                            phat[:, 0, reg],
                            start=(j == 0),
                            stop=(j == nblk - 1),
                        )
                        nc.tensor.matmul(
                            ps_pv_o[:, reg],
                            va_t[:, j, 2 * hp + 1, :],
                            phat[:, 1, reg],
                            start=(j == 0),
                            stop=(j == nblk - 1),
                        )
                    # Normalize via an SBUF staging copy of the whole PV
                    # accumulator (values + den row): the single DVE copy is
                    # the only PSUM reader, so the pv ring frees ~3x sooner
                    # for the next hp, and the reciprocal runs from SBUF (the
                    # custom recip uop reading PSUM misbehaves on HW).
                    # The very last normalize feeds the tail out-projection,
                    # so it runs in 128-column slices to shorten the critical
                    # path into outproj's first stripe.
                    # The den row must reach a partition-0 SBUF tile via an
                    # ACT copy before the reciprocal: the custom recip uop is
                    # only correct for partition-0 SBUF inputs on HW. The
                    # values rows are staged to SBUF by one DVE copy, whose
                    # early completion releases the pv ring for the next hp.
                    # The very last normalize feeds the tail out-projection,
                    # so it runs in 128-column slices (no staging needed:
                    # nothing reuses the pv ring after it).
                    tailcase = causal and i == NI - 1 and hp == 1
                    pvs = (ps_pv_e, ps_pv_o)
                    if not tailcase:
                        stages = []
                        for sl2 in range(2):
                            stage = normpool.tile(
                                [HD, 512], F32, tag="stage", name="stage"
                            )
                            nc.vector.tensor_copy(stage[:], pvs[sl2][0:HD, :])
                            stages.append(stage)
                    col_slices = (
                        [slice(0, 256), slice(256, 512)]
                        if tailcase
                        else [slice(0, 512)]
                    )
                    for csl in col_slices:
                        w = csl.stop - csl.start
                        recs = []
                        for sl2 in range(2):
                            den = normpool.tile([1, 512], F32, tag="den", name="den")
                            nc.scalar.copy(den[:, 0:w], pvs[sl2][HD : HD + 1, csl])
                            rec1 = normpool.tile([1, 512], F32, tag="rec1", name="rec1")
                            nc.vector.reciprocal_approx_fast(rec1[:, 0:w], den[:, 0:w])
                            recs.append(rec1)
                        brecs = []
                        for sl2 in range(2):
                            rec = normpool.tile([64, 512], F32, tag="rec", name="rec")
                            nc.gpsimd.partition_broadcast(
                                rec[:, 0:w], recs[sl2][:, 0:w]
                            )
                            brecs.append(rec)
                        for sl2 in range(2):
                            nc.vector.tensor_mul(
                                ao[hp][64 * sl2 : 64 * sl2 + 64, csl],
                                pvs[sl2][0:HD, csl] if tailcase else stages[sl2][:, csl],
                                brecs[sl2][:, 0:w],
                            )
                    if hp == 0:
                        pop_fill(extra=2)
                while fill:
                    fill.pop(0)[0]()
                return ao

            # causal: software-pipelined per-chunk schedule. attention(i) only
            # needs kT/v for t-blocks <= chunk end, so chunk i+1's projection
            # and chunk i-1's out-projection are interleaved into attention(i)
            # as PE fill work. non-causal: attention needs the full kT/v, so
            # project everything first.
            for _rep in range(reps):
                if causal:
                    xq0 = proj_chunk0()
                    vproj(0, xq0)
                    ao_all = []
                    xq_boxes = [[None] for _ in range(NI + 1)]
                    for i in range(NI):
                        fill = []
                        if i + 1 < NI:
                            emit_loads(i + 1, xq_boxes[i + 1])
                            fill += proj_closures(i + 1, xq_boxes[i + 1])
                        if i == NI - 2:
                            # chunk 0's out-projection fills attention(2),
                            # which has projection fill to spare
                            fill += outproj_closures(0, ao_all[0], dve_only=True)
                        if i == NI - 1:
                            # the rest fill the largest attention chunk,
                            # which has no projection left
                            for k in range(1, NI - 1):
                                fill += outproj_closures(k, ao_all[k], dve_only=True)
                        ao_all.append(
                            attention(i, fill, pop_delay=3 if i + 1 < NI else 0)
                        )
                        if i + 1 < NI:
                            vproj(i + 1, xq_boxes[i + 1][0])
                    for c, _ in outproj_closures(NI - 1, ao_all[-1]):
                        c()
                else:
                    xq0 = proj_chunk0()
                    vproj(0, xq0)
                    xq_boxes = [[None] for _ in range(NI)]
                    for i in range(1, NI):
                        emit_loads(i, xq_boxes[i])
                        for c, _ in proj_closures(i, xq_boxes[i]):
                            c()
                        vproj(i, xq_boxes[i][0])
                    ao_prev = None
                    for i in range(NI):
                        fill = (
                            outproj_closures(i - 1, ao_prev)
                            if ao_prev is not None
                            else []
                        )
                        ao_prev = attention(i, fill)
                    for c, _ in outproj_closures(NI - 1, ao_prev):
                        c()

    nc.compile()
    return nc


# --------------------------------------------------------------------------
# host-side: constants, sharding, assembly
# --------------------------------------------------------------------------

def _rope_tables():
    inv_freq = 1.0 / (
        ROPE_BASE ** (np.arange(0, HD, 2, dtype=np.float32) / HD)
    )
    positions = np.arange(S, dtype=np.float32)
    freqs = np.outer(positions, inv_freq).astype(np.float32)     # [S, 32]
    emb = np.concatenate((freqs, freqs), axis=-1)                # [S, 64]
    cosT = np.cos(emb).T.astype(np.float32)                      # [64, S]
    sinT = np.sin(emb).T.astype(np.float32)
    # fold the rotate_half sign into sin: rot[2i] = -x[2i+1] -> sin'[2i] < 0
    sgn = np.where(np.arange(HD) % 2 == 0, -1.0, 1.0)[:, None].astype(np.float32)
    sinT = sinT * sgn
    cos2 = np.vstack([cosT, cosT]).astype(NPBF)                  # [128, S]
    sin2 = np.vstack([sinT, sinT]).astype(NPBF)
    return cos2, sin2


_CACHE: dict = {}


def _get_nc(causal: bool):
    key = ("nc", causal)
    if key not in _CACHE:
        _CACHE[key] = build_nc(causal)
    return _CACHE[key]


def _classify_mask(mask: np.ndarray) -> str:
    m = np.asarray(mask).reshape(S, S)
    if np.array_equal(m != 0, np.tril(np.ones((S, S), bool))):
        return "causal"
    if np.all(m != 0):
        return "full"
    return "other"


def make_in_maps(x, w_qkv, w_out):
    """Build the 8 per-core input dicts (device tensors in bf16)."""
    cos2, sin2 = _rope_tables()
    # mask01[t, b, s''] = 0 where t > s'' (strict lower triangle masked), for
    # both head slots b
    m01 = (np.arange(128)[:, None] <= np.arange(128)[None, :]).astype(np.float32)
    mask01 = np.ascontiguousarray(
        np.stack([m01, m01], axis=1).reshape(128, 256)
    ).astype(NPBF)

    w3 = np.asarray(w_qkv).reshape(D, 3, H, HD)   # [D, {q,k,v}, H, hd]
    wo = np.asarray(w_out)                        # [D, D]; rows indexed [h, hd]
    xT = [np.ascontiguousarray(np.asarray(x)[b].T).astype(NPBF) for b in range(B)]

    in_maps = []
    for c in range(8):
        b, hg = divmod(c, 4)
        hs = [4 * hg + i for i in range(HPC)]
        # w_qk cols: [q_h0, q_h1, k_h0, k_h1, q_h2, q_h3, k_h2, k_h3]
        wqk_cols = []
        for pair in range(2):
            for t in range(2):  # 0 = q, 1 = k
                for hh in (hs[2 * pair], hs[2 * pair + 1]):
                    wqk_cols.append(w3[:, t, hh, :])
        w_qk_c = np.ascontiguousarray(np.concatenate(wqk_cols, axis=1)).astype(NPBF)
        w_v_c = np.ascontiguousarray(
            np.concatenate([w3[:, 2, hh, :] for hh in hs], axis=1)
        ).astype(NPBF)  # [D, 256]
        w_out_c = np.ascontiguousarray(
            np.concatenate([wo[HD * hh : HD * hh + HD, :] for hh in hs], axis=0)
        ).astype(NPBF)  # [256, D]
        in_maps.append(
            {
                "xT": xT[b],
                "w_qk": w_qk_c,
                "w_v": w_v_c,
                "w_out": w_out_c,
                "cos2": cos2,
                "sin2": sin2,
                "mask01": mask01,
            }
        )
    return in_maps


def _reference_numpy(x, mask, w_qkv, w_out):
    """Exact fallback for non-causal, non-full masks (slow, host-side)."""
    x = np.asarray(x, np.float32)
    qkv = (x @ w_qkv).reshape(B, S, 3, H, HD)
    qkv = np.transpose(qkv, (2, 0, 3, 1, 4))
    q, k, v = qkv[0], qkv[1], qkv[2]
    inv_freq = 1.0 / (ROPE_BASE ** (np.arange(0, HD, 2, dtype=np.float32) / HD))
    positions = np.arange(S, dtype=np.float32)
    freqs = np.outer(positions, inv_freq).astype(np.float32)
    emb = np.concatenate((freqs, freqs), axis=-1)
    cos = np.cos(emb)[None, None]
    sin = np.sin(emb)[None, None]

    def rot(t):
        t1 = t[..., ::2]
        t2 = t[..., 1::2]
        return np.stack((-t2, t1), axis=-1).reshape(t.shape)

    q = q * cos + rot(q) * sin
    k = k * cos + rot(k) * sin
    attn = np.einsum("bhsd,bhtd->bhst", q, k) / math.sqrt(HD)
    m = np.asarray(mask).reshape(1, 1, S, S)
    attn = np.where(m == 0, -np.inf, attn)
    attn = attn - attn.max(-1, keepdims=True)
    np.exp(attn, out=attn)
    attn /= attn.sum(-1, keepdims=True)
    out = np.einsum("bhst,bhtd->bhsd", attn, v)
    out = np.transpose(out, (0, 2, 1, 3)).reshape(B, S, D)
    return (out @ w_out).astype(np.float32)


class Runner:
    """Cached jitted SPMD runner (mirrors bass2jax.run_bass_via_pjrt)."""

    def __init__(self, nc, n_cores: int = 8):
        import jax
        import concourse.mybir as _mybir
        from concourse import bass2jax
        from jax.experimental.shard_map import shard_map
        from jax.sharding import Mesh, PartitionSpec

        bass2jax.install_neuronx_cc_hook()
        self.jax = jax
        self.n_cores = n_cores
        self._nc = nc
        in_names, out_names, out_avals, zero_outs = [], [], [], []
        for alloc in nc.m.functions[0].allocations:
            if not isinstance(alloc, _mybir.MemoryLocationSet):
                continue
            name = alloc.memorylocations[0].name
            if alloc.kind == "ExternalInput":
                in_names.append(name)
            elif alloc.kind == "ExternalOutput":
                out_names.append(name)
                shape = tuple(alloc.tensor_shape)
                dtype = _mybir.dt.np(alloc.dtype)
                out_avals.append(jax.core.ShapedArray(shape, dtype))
                zero_outs.append(np.zeros(shape, dtype))
        self.in_names = list(in_names)
        self.out_names = out_names
        self.out_avals = out_avals
        self.zero_outs = zero_outs
        all_names = in_names + out_names

        def _body(*args):
            outs = bass2jax._bass_exec_p.bind(
                *args,
                out_avals=tuple(out_avals),
                in_names=tuple(all_names),
                out_names=tuple(out_names),
                lowering_input_output_aliases=(),
                sim_require_finite=True,
                sim_require_nnan=True,
                nc=nc,
            )
            return tuple(outs)

        devices = jax.devices()[:n_cores]
        self.mesh = Mesh(np.asarray(devices), ("core",))
        n_args = len(all_names)
        self.sharded = jax.jit(
            shard_map(
                _body,
                mesh=self.mesh,
                in_specs=(PartitionSpec("core"),) * n_args,
                out_specs=(PartitionSpec("core"),) * len(out_names),
                check_rep=False,
            )
        )

    def concat_inputs(self, in_maps):
        cols = []
        for name in self.in_names:
            if name == "partition_id":
                cols.append(
                    np.arange(self.n_cores, dtype=np.uint32).reshape(
                        self.n_cores, 1
                    )
                )
            else:
                cols.append(
                    np.concatenate([np.asarray(m[name]) for m in in_maps], axis=0)
                )
        return cols

    def device_put(self, concat_in):
        """Place concatenated inputs (and zero output buffers) on the mesh."""
        from jax.sharding import NamedSharding, PartitionSpec

        sh = NamedSharding(self.mesh, PartitionSpec("core"))
        args = concat_in + [
            np.zeros((self.n_cores * z.shape[0], *z.shape[1:]), z.dtype)
            for z in self.zero_outs
        ]
        return [self.jax.device_put(a, sh) for a in args]

    def run_dev(self, dev_args):
        return self.sharded(*dev_args)

    def make_bench(self, n_reps: int):
        """Jitted fn executing the NEFF n_reps times serially on-device."""
        import jax
        from concourse import bass2jax
        from jax.experimental.shard_map import shard_map
        from jax.sharding import Mesh, PartitionSpec

        nc = self._nc
        out_avals = self.out_avals
        all_names = self.in_names + self.out_names
        out_names = self.out_names

        def _body(*args):
            outs = None
            for _ in range(n_reps):
                outs = bass2jax._bass_exec_p.bind(
                    *args,
                    out_avals=tuple(out_avals),
                    in_names=tuple(all_names),
                    out_names=tuple(out_names),
                    lowering_input_output_aliases=(),
                    sim_require_finite=True,
                    sim_require_nnan=True,
                    nc=nc,
                )
            return tuple(outs)

        n_args = len(all_names)
        return jax.jit(
            shard_map(
                _body,
                mesh=self.mesh,
                in_specs=(PartitionSpec("core"),) * n_args,
                out_specs=(PartitionSpec("core"),) * len(out_names),
                check_rep=False,
            )
        )

    def run(self, in_maps):
        dev_args = self.device_put(self.concat_inputs(in_maps))
        out_arrs = self.sharded(*dev_args)
        outs = []
        for c in range(self.n_cores):
            outs.append(
                {
                    name: np.asarray(out_arrs[i]).reshape(
                        self.n_cores, *self.out_avals[i].shape
                    )[c]
                    for i, name in enumerate(self.out_names)
                }
            )
        return outs


def _get_runner(causal: bool) -> Runner:
    key = ("runner", causal)
    if key not in _CACHE:
        _CACHE[key] = Runner(_get_nc(causal))
    return _CACHE[key]


def run_spmd(in_maps, causal: bool = True, **kw):
    nc = _get_nc(causal)
    return run_bass_kernel_spmd(nc, in_maps, core_ids=list(range(8)), **kw)


def kernel(x, mask, w_qkv, w_out):
    kind = _classify_mask(mask)
    if kind == "other":
        return _reference_numpy(x, mask, w_qkv, w_out)
    in_maps = make_in_maps(x, w_qkv, w_out)
    res = run_spmd(in_maps, causal=(kind == "causal"))
    out = np.zeros((B, S, D), np.float32)
    for c in range(8):
        out[c // 4] += res.results[c]["outp"]
    return out


if __name__ == "__main__":
    rng = np.random.default_rng(0)
    x = rng.standard_normal((B, S, D)).astype(np.float32)
    mask = np.tril(np.ones((S, S), np.int32)).reshape(1, 1, S, S)
    w_qkv = (rng.standard_normal((D, 3 * D)) * 0.02).astype(np.float32)
    w_out = (rng.standard_normal((D, D)) * 0.02).astype(np.float32)
    got = kernel(x, mask, w_qkv, w_out)
    print("kernel ran, out shape", got.shape)
